# revision 23
# baseline (speedup 1.0000x reference)
"""Trainium2 Bass kernel for the fused L2-embed / RMS-norm / tanh-gate module.

  sumsq[n,c] = sum_{h,w} x[n,c,h,w]^2
  embed      = sqrt(sumsq + eps) * alpha
  inv[n]     = rsqrt(mean_c(embed^2) + eps)
  z          = embed * gamma * inv + beta
  out        = x * (1 + tanh(z))

Data-parallel over the batch axis: 8 samples per NeuronCore, 8 cores.

The kernel is HBM-bound (in+out share a ~435 GB/s per-core DMA budget), so x
is streamed in and the result streamed out as fp16 (the harness gate is
rel_err < 2e-2; fp16 I/O contributes ~3e-4).  All 8 fp16 samples fit in SBUF
(12.6 MB).  Every x transfer rides ONE HWDGE ring (sync/SP): 16 half-sample
loads first, 8 sample stores behind them in FIFO order, so the ring streams
loads at full rate, then stores, with no cross-ring round-robin dilution and
no descriptor generation on the compute engines.  The tiny param loads ride
the scalar ring, landing ~6us in, well before stage B needs them.

Compute is paced so it hides entirely under the DMA stream: ScalarE squares
the k=0 half of samples 0-5 and both halves of samples 6-7 (ACT Square +
accumulator), VectorE squares k=1 of samples 0-5 (scalar_tensor_tensor with
sum accumulator), stage B runs per 2-sample batch in a k-major column
layout, and the fp16 gate apply hits the DVE 4x path (~1.03us per half).
Explicit add_dep_helper edges keep the Tile scheduler from hoisting all
squares ahead of the tanh/gate chain (which would starve the store stream).
"""

import json

import numpy as np

N, C, H, W = 64, 256, 56, 56
HW = H * W                    # 3136
NCORES = 8
NPC = N // NCORES             # samples per core
EPS = 1e-5
P = 128
K = C // P                    # free-dim channel halves per partition (2)
NB = 2                        # samples per stage-B batch
NBATCH = NPC // NB
RSQRT_MAGIC = 0x5F3759DF
ACT_SQ_SAMPLES = (2, 3, 4, 5)  # samples whose k=1 square also runs on ScalarE
# (mid-kernel samples: ACT has an idle window there, while the first and last
# samples' k=1 squares stay on VectorE so the pipeline head starts fast and
# the tail's sumsq finishes on both engines in parallel; VectorE carries the
# gates, so keeping its queue short pulls every store ahead of its ring slot)

_cache = {}


# --------------------------------------------------------------------------
# BIR post-processing: the walrus build in this container allows at most one
# sync wait and one sync update per instruction.  Hoist excess waits onto
# NoOps inserted before the instruction (same engine/block); move excess
# updates of non-DMA instructions onto a NoOp right after.
# --------------------------------------------------------------------------
_nop_counter = [0]


def _mk_nop(engine, waits, updates, debug=0):
    _nop_counter[0] += 1
    return {
        "name": f"I-wsplit-{_nop_counter[0]}",
        "opcode": "NoOp",
        "engine": engine,
        "ins": [],
        "outs": [],
        "debug": debug,
        "sync_info": {"on_wait": waits, "on_update": updates},
    }


def _split_sync_waits(bir_json_bytes):
    d = json.loads(bir_json_bytes)
    for f in d.get("functions", []):
        for blk in f.get("blocks", []):
            new_insts = []
            for inst in blk.get("instructions", []):
                si = inst.get("sync_info")
                after = []
                if si:
                    waits = list(si.get("on_wait") or [])
                    updates = list(si.get("on_update") or [])
                    eng = inst.get("engine")
                    dbg = inst.get("debug", 0)
                    if len(waits) > 1:
                        for w in waits[:-1]:
                            new_insts.append(_mk_nop(eng, [w], [], dbg))
                        waits = waits[-1:]
                    if len(updates) > 1:
                        op = inst.get("opcode", "")
                        if "DMA" in op:
                            raise RuntimeError(
                                f"DMA instruction {inst.get('name')} has "
                                f"{len(updates)} sync updates; cannot split"
                            )
                        for u in updates[1:]:
                            after.append(_mk_nop(eng, [], [u], dbg))
                        updates = updates[:1]
                    si["on_wait"] = waits
                    si["on_update"] = updates
                new_insts.append(inst)
                new_insts.extend(after)
            blk["instructions"] = new_insts
    return json.dumps(d).encode()


def _patch_bass(nc):
    orig = nc.to_json_bytes

    def fixed(*a, **kw):
        return _split_sync_waits(orig(*a, **kw))

    nc.to_json_bytes = fixed
    return nc


# --------------------------------------------------------------------------
# Kernel build
# --------------------------------------------------------------------------
def _build():
    import bass_rust
    import concourse.bass as bass
    import concourse.tile as tile
    from concourse import mybir
    from concourse.tile import ScopedClock

    f16 = mybir.dt.float16
    f32 = mybir.dt.float32
    u32 = mybir.dt.uint32
    Alu = mybir.AluOpType
    Act = mybir.ActivationFunctionType

    def dep(after_inst, before_inst, why):
        """Scheduler-ordering edge: after_inst must come after before_inst."""
        bass_rust.add_dep_helper(
            after_inst.ins, before_inst.ins, sync=False, reason=why
        )

    class LeanExitTileContext(tile.TileContext):
        """Standard exit minus both all-engine barriers (~3us each).
        The sem clears sit on gpsimd's own stream behind a gpsimd drain that
        waits on the global clock (all sems at final values), so they cannot
        run early; every other engine's stream simply ends, and NRT only
        starts a subsequent execution after every stream has ended."""

        def _drain_and_barrier(self, tick_clock, wait_clock):
            drain_inst = self.nc.gpsimd.drain()
            wait_clock.add_sem_waits(
                drain_inst.ins, ScopedClock({None: tick_clock.global_clock})
            )
            assert self.sems is not None
            popped = self.nc._tile_sem_poison_stack.pop()
            assert popped is self._sem_poison
            self.nc.clear_and_free_semaphores(
                list(self.sems.allocated().values())
            )

    nc = bass.Bass(trn_type="TRN2")
    x = nc.dram_tensor("x", [NPC, C, HW], f16, kind="ExternalInput")
    alpha = nc.dram_tensor("alpha", [C], f32, kind="ExternalInput")
    gamma = nc.dram_tensor("gamma", [C], f32, kind="ExternalInput")
    beta = nc.dram_tensor("beta", [C], f32, kind="ExternalInput")
    out = nc.dram_tensor("out", [NPC, C, HW], f16, kind="ExternalOutput")

    with LeanExitTileContext(nc) as tc:
        with (
            tc.tile_pool(name="xpool", bufs=NPC) as xpool,
            tc.tile_pool(name="scratch", bufs=1) as scratch,
            tc.tile_pool(name="small", bufs=4) as small,
            tc.tile_pool(name="singles", bufs=1) as singles,
            tc.tile_pool(name="ps", bufs=4, space="PSUM") as ps,
        ):
            # ---- all 16 half-sample loads up front on the sync/SP ring, in
            # sample-major order so sample n is fully resident ~3.6us after
            # sample n-1.  One ring saturates the fabric on its own; keeping
            # the scalar(ACT) engine free of x-descgen lets squares start
            # as early as possible. ----
            xts = []
            load_insts = []
            prev = None
            for n in range(NPC):
                xt = xpool.tile([P, K, HW], f16)
                xv = x[n].rearrange("(p a) hw -> p a hw", p=P)
                for k in range(K):
                    di = nc.sync.dma_start(out=xt[:, k], in_=xv[:, k])
                    if prev is not None:
                        dep(di, prev, "x loads stream in sample order")
                    prev = di
                    load_insts.append(di)
                xts.append(xt)
            last_load = load_insts[-1]

            # ---- tiny param loads ride gpsimd SWDGE: its Q7 preamble means
            # they land ~13-20us in, but nothing needs them before stage B of
            # batch 0 (~21us), and this keeps both HWDGE-issuing engines
            # (sync: x descgen, scalar: squares) free of the slow 128-
            # descriptor scatter.  channel c -> (partition c//K, half c%K).
            a_col = singles.tile([P, K], f32)
            nc.gpsimd.dma_start(out=a_col[:], in_=alpha[:].rearrange("(p a) -> p a", p=P))
            g_col = singles.tile([P, K], f32)
            nc.gpsimd.dma_start(out=g_col[:], in_=gamma[:].rearrange("(p a) -> p a", p=P))
            b_col = singles.tile([P, K], f32)
            nc.gpsimd.dma_start(out=b_col[:], in_=beta[:].rearrange("(p a) -> p a", p=P))
            zero_bias = singles.tile([P, 1], f32)  # memset, not const-DMA:
            nc.vector.memset(zero_bias[:], 0.0)    # keeps ACT off the const
            # tensor DMA dependency that otherwise delays the first square

            ones_nb = singles.tile([P, NB], f32)
            nc.vector.memset(ones_nb[:], 1.0)
            ones_t = singles.tile([P, P], f32)       # all-ones lhsT for col-sum
            nc.vector.memset(ones_t[:], 1.0)
            magic = singles.tile([P, K * NB], u32)   # rsqrt seed constant
            nc.vector.memset(magic[:], RSQRT_MAGIC)

            # Param-derived one-time tiles.  These WAIT on the slow gpsimd
            # param DMAs, so they are pinned (below) behind batch 0's DVE
            # squares — otherwise they sit first in the DVE queue and stall
            # it until the params land (cost ~9us in a previous revision).
            one_time = []

            a2_col = singles.tile([P, K], f32)       # alpha^2
            one_time.append(nc.vector.tensor_mul(a2_col[:], a_col[:], a_col[:]))
            ag_col = singles.tile([P, K], f32)       # alpha*gamma
            one_time.append(nc.vector.tensor_mul(ag_col[:], a_col[:], g_col[:]))

            # k-major batched param tiles [P, K*NB]: column k*NB+j holds the
            # (partition, k) channel's value for batch-sample j.
            a2_b = singles.tile([P, K * NB], f32)
            ag_b = singles.tile([P, K * NB], f32)
            b_b = singles.tile([P, K * NB], f32)
            for k in range(K):
                one_time.append(nc.vector.tensor_scalar_mul(
                    a2_b[:, k * NB : (k + 1) * NB], ones_nb[:], a2_col[:, k : k + 1]
                ))
                one_time.append(nc.vector.tensor_scalar_mul(
                    ag_b[:, k * NB : (k + 1) * NB], ones_nb[:], ag_col[:, k : k + 1]
                ))
                one_time.append(nc.vector.tensor_scalar_mul(
                    b_b[:, k * NB : (k + 1) * NB], ones_nb[:], b_col[:, k : k + 1]
                ))

            # reused dummy outputs for the square passes (values unused,
            # only accum_out matters); WAW lands on the same serial engine.
            sq_act = scratch.tile([P, HW], f16)
            sq_dve = scratch.tile([P, HW], f16)

            first_act_sq = [None] * NBATCH   # batch -> first ACT square
            first_smalls = [None] * NBATCH   # batch -> first stage-B DVE op
            last_dve_sq = [None] * NBATCH    # batch -> last DVE square
            tanh_of = [None] * NBATCH
            last_gate = [None] * NBATCH

            for b in range(NBATCH):
                ns = [b * NB + j for j in range(NB)]
                S = small.tile([P, K * NB], f32)     # sumsq, col = k*NB+j

                # ---- stage A: per-channel sumsq. ScalarE takes k=0 (plus
                # both halves of the tail samples, where ACT is otherwise
                # idle), VectorE takes the rest of k=1 via STT + accum. ----
                for j, n in enumerate(ns):
                    ai = nc.scalar.activation(
                        out=sq_act[:],
                        in_=xts[n][:, 0],
                        func=Act.Square,
                        bias=zero_bias[:, 0:1],
                        accum_out=S[:, 0 * NB + j : 0 * NB + j + 1],
                    )
                    if first_act_sq[b] is None:
                        first_act_sq[b] = ai
                    if n in ACT_SQ_SAMPLES:
                        nc.scalar.activation(
                            out=sq_act[:],
                            in_=xts[n][:, 1],
                            func=Act.Square,
                            bias=zero_bias[:, 0:1],
                            accum_out=S[:, 1 * NB + j : 1 * NB + j + 1],
                        )
                    else:
                        vi = nc.vector.scalar_tensor_tensor(
                            out=sq_dve[:],
                            in0=xts[n][:, 1],
                            scalar=1.0,
                            in1=xts[n][:, 1],
                            op0=Alu.mult,
                            op1=Alu.mult,
                            accum_out=S[:, 1 * NB + j : 1 * NB + j + 1],
                        )
                        last_dve_sq[b] = vi
                        if b >= 1 and last_gate[b - 1] is not None:
                            # keep DVE squares of this batch behind the
                            # previous batch's gates: the gates feed the
                            # store stream, the squares have slack
                            dep(
                                vi,
                                last_gate[b - 1],
                                "DVE: prior gates before this batch's squares",
                            )

                # ---- stage B for the batch (tiny [P, K*NB] ops) ----
                # u = sumsq + eps ; ua = u * alpha^2  (= embed^2)
                u_t = small.tile([P, K * NB], f32)
                ui = nc.vector.tensor_scalar(u_t[:], S[:], EPS, None, op0=Alu.add)
                first_smalls[b] = ui
                ua = small.tile([P, K * NB], f32)
                nc.vector.tensor_mul(ua[:], u_t[:], a2_b[:])

                # col-sum of embed^2 broadcast to all partitions via PE;
                # the two k-halves accumulate into one PSUM tile so the
                # per-sample sum needs no extra DVE op.
                ms = ps.tile([P, NB], f32)
                nc.tensor.matmul(ms[:], ones_t[:], ua[:, 0:NB], start=True, stop=False)
                nc.tensor.matmul(
                    ms[:], ones_t[:], ua[:, NB : 2 * NB], start=False, stop=True
                )

                # v = mean + eps ; w[:, k-block] = u / v
                v_t = small.tile([P, NB], f32)
                nc.vector.tensor_scalar(
                    v_t[:], ms[:], 1.0 / C, EPS, op0=Alu.mult, op1=Alu.add
                )
                rv = small.tile([P, NB], f32)
                nc.vector.reciprocal(rv[:], v_t[:])
                w_t = small.tile([P, K * NB], f32)
                for k in range(K):
                    nc.vector.tensor_mul(
                        w_t[:, k * NB : (k + 1) * NB],
                        u_t[:, k * NB : (k + 1) * NB],
                        rv[:],
                    )

                # y ~= rsqrt(w): bit-trick seed + 1 Newton iteration
                # (seed err ~3.4e-2 -> ~1.7e-3 after one iteration; the
                # output is fp16 and the harness gate is 2e-2, so the extra
                # iteration would only burn VectorE time on the gate path)
                y_t = small.tile([P, K * NB], f32)
                sh = small.tile([P, K * NB], u32)
                nc.vector.tensor_scalar(
                    sh[:], w_t[:].bitcast(u32), 1, None, op0=Alu.logical_shift_right
                )
                nc.vector.tensor_tensor(
                    out=y_t[:].bitcast(u32), in0=magic[:], in1=sh[:], op=Alu.subtract
                )
                t_t = small.tile([P, K * NB], f32)
                for _ in range(1):
                    nc.vector.tensor_mul(t_t[:], w_t[:], y_t[:])
                    nc.vector.tensor_mul(t_t[:], t_t[:], y_t[:])
                    nc.vector.tensor_scalar(
                        t_t[:], t_t[:], -0.5, 1.5, op0=Alu.mult, op1=Alu.add
                    )
                    nc.vector.tensor_mul(y_t[:], y_t[:], t_t[:])

                # z = alpha*gamma*sqrt(w) + beta ;  sqrt(w) = w * rsqrt(w)
                z_t = small.tile([P, K * NB], f32)
                nc.vector.tensor_mul(z_t[:], w_t[:], y_t[:])
                nc.vector.tensor_mul(z_t[:], z_t[:], ag_b[:])
                nc.vector.tensor_add(z_t[:], z_t[:], b_b[:])

                # gate = 1 + tanh(z)   (tanh is the only ACT table user)
                gt = small.tile([P, K * NB], f32)
                ti = nc.scalar.activation(
                    out=gt[:], in_=z_t[:], func=Act.Tanh, bias=zero_bias[:, 0:1]
                )
                tanh_of[b] = ti
                nc.vector.tensor_scalar(gt[:], gt[:], 1.0, None, op0=Alu.add)

                # ---- apply gate in-place (fp16 4x path), stream each sample
                # out as a single 1.6MB store on the sync ring, FIFO behind
                # the loads. ----
                for j, n in enumerate(ns):
                    out_n = out[n].rearrange("(p a) hw -> p a hw", p=P)
                    for k in range(K):
                        gi = nc.vector.tensor_scalar_mul(
                            xts[n][:, k],
                            in0=xts[n][:, k],
                            scalar1=gt[:, k * NB + j : k * NB + j + 1],
                        )
                        last_gate[b] = gi
                        si = nc.sync.dma_start(out=out_n[:, k], in_=xts[n][:, k])
                        dep(si, last_load, "stores queue behind all loads")

            # ---- scheduler fences: without these the priority heap hoists
            # every square ahead of the tanh/gate chain, so stores only start
            # ~25us after the loads drain and the DMA ring sits idle.  The
            # stage-B+gate chain runs strictly batch by batch (squares may
            # still float to fill engine gaps); tanh gets a two-batch window
            # on ACT so squares keep ACT busy while DVE prepares z. ----
            for b in range(1, NBATCH):
                dep(
                    first_smalls[b],
                    last_gate[b - 1],
                    "DVE: gates of batch b-1 before stage B of batch b",
                )
            for b in range(2, NBATCH):
                dep(
                    first_act_sq[b],
                    tanh_of[b - 2],
                    "ACT: tanh of batch b-2 before squares of batch b",
                )

            # Pin the param-derived one-time ops behind batch 0's DVE
            # squares: they wait on the slow gpsimd param DMAs and must not
            # block the front of the DVE queue.
            for ot in one_time:
                dep(
                    ot,
                    last_dve_sq[0],
                    "param-derived tiles wait behind batch-0 DVE squares",
                )

    return _patch_bass(nc)


def _get_nc():
    if "nc" not in _cache:
        _cache["nc"] = _build()
    return _cache["nc"]


def _ensure_axon_hooks_stub():
    """bass_utils imports antenv.axon_hooks when tracing is requested (e.g.
    via a stray BASS_TRACE=1); this image lacks that module. Provide a stub
    whose hook getter returns None so the untraced fallback path runs."""
    import sys
    import types

    try:
        import antenv.axon_hooks  # noqa: F401
    except ImportError:
        mod = types.ModuleType("antenv.axon_hooks")
        _holder = [None]
        mod.set_axon_ntff_profile_hook = lambda h: _holder.__setitem__(0, h)
        mod.get_axon_ntff_profile_hook = lambda: _holder[0]
        sys.modules["antenv.axon_hooks"] = mod


def _run(x, alpha, gamma, beta, trace=False, **spmd_kwargs):
    from concourse.bass_utils import run_bass_kernel_spmd

    _ensure_axon_hooks_stub()

    nc = _get_nc()
    x = np.asarray(x)
    x16 = np.ascontiguousarray(x, dtype=np.float16).reshape(N, C, HW)
    alpha = np.ascontiguousarray(np.asarray(alpha), dtype=np.float32)
    gamma = np.ascontiguousarray(np.asarray(gamma), dtype=np.float32)
    beta = np.ascontiguousarray(np.asarray(beta), dtype=np.float32)
    in_maps = [
        {
            "x": np.ascontiguousarray(x16[c * NPC : (c + 1) * NPC]),
            "alpha": alpha,
            "gamma": gamma,
            "beta": beta,
        }
        for c in range(NCORES)
    ]
    res = run_bass_kernel_spmd(
        nc, in_maps, core_ids=list(range(NCORES)), trace=trace, **spmd_kwargs
    )
    full = np.concatenate([r["out"] for r in res.results], axis=0)
    return full.reshape(N, C, H, W).astype(np.float32), res


def kernel(x, alpha, gamma, beta):
    out, _ = _run(x, alpha, gamma, beta)
    return out
